# revision 13
# baseline (speedup 1.0000x reference)
"""2-layer GCN (GCNConv -> ReLU -> GCNConv) on 8 Trainium2 NeuronCores. v3.

conv = dinv * (sum_{src->dst} y'[src] + y'[dst]) + b with y' = dinv*(x@W).

v3 gather core (vs v2): the table is stored in LAYOUT order (nodes permuted
by degree-desc into (core, chunk, slot) positions), so
 - self-loop contributions are a sequential DRAM read (no gather call),
 - per-chunk calls use a ragged "conjugate staircase": call k covers only the
   first n_k slots (slots sorted by degree desc), n_k exact to 16 -> the
   descriptor count equals the edge count (~0.15% waste) instead of padding
   every call to 1024 slots.
DVE work per call is 2 ops (masked-mult into tm, wide f32 accumulate); the
quad-select reduce runs once per chunk instead of once per call.
Gathers are InstDMAGatherAnt (128B quad rows, int16 quad indices), round-robin
over 4 SWDGE queues -- measured ~8.6ns/descriptor/queue, perfectly
queue-parallel, element-size independent (descriptor-rate-bound).

Raw Block style; engines sync via explicit cumulative semaphores.
"""

import os
import sys

for _p in ("/opt/trn_rl_repo", "/root/.axon_site/_ro/trn_rl_repo"):
    if os.path.isdir(_p) and _p not in sys.path:
        sys.path.append(_p)

from contextlib import ExitStack

import numpy as np
import ml_dtypes

import concourse.bass as bass
import concourse.bacc as bacc
import concourse.tile as tile
from concourse import mybir
from concourse import ap_utils
from concourse.bass import MemorySpace
from concourse.bass_utils import run_bass_kernel_spmd

dt = mybir.dt
F32 = dt.float32
BF16 = dt.bfloat16
I16 = dt.int16
ALU = mybir.AluOpType
AXL = mybir.AxisListType

N = 100000          # real nodes
F = 256             # input features
H = 16              # hidden
O = 40              # classes
NCORES = 8
P = 128
C = 8               # node columns per partition per chunk
CH = P * C          # 1024 nodes per chunk
CHUNKS = 13
PC = CH * CHUNKS    # 13312 nodes per core
NPAD = PC * NCORES  # 106496 padded node space
QROWS = NPAD // 4   # quad rows in gather tables
ZRQ = QROWS - 1     # last quad row = padding nodes = zeros
NQ = 4              # SWDGE queues
B = 48              # gather buffer ring
RS = 13             # rotating sems per queue; NQ*RS > B

_TRACE = bool(os.environ.get("GNN_TRACE"))
_EXEC_NS = []


def dma_gather_raw(g, out_ap, in_ap, idxs_ap, num_idxs, num_idxs_reg,
                   elem_size, elem_step, queue_num):
    """bass dma_gather minus the elem_size%256 assert (non-transpose ucode
    supports any elem size; only the row stride must be a 256B multiple)."""
    assert idxs_ap.dtype == mybir.dt.int16
    assert in_ap.dtype == out_ap.dtype
    assert in_ap.space == MemorySpace.DRAM
    assert ap_utils.ap_is_contiguous(in_ap.ap[1:])
    assert ap_utils.ap_is_contiguous(out_ap.ap[1:])
    assert ap_utils.ap_is_contiguous(idxs_ap.ap[1:])
    assert in_ap.ap[-1][1] == out_ap.ap[-1][1] == elem_size
    assert in_ap.ap[0][0] == elem_step
    stride_bytes = elem_step * mybir.dt.size(in_ap.dtype)
    stride_bytes_256 = stride_bytes // 256
    assert stride_bytes_256 * 256 == stride_bytes and stride_bytes_256 < 256
    _in_ap = g.lower_ap_dma(in_ap, for_custom_bir_dma=True)
    _idxs_ap = g.lower_ap(idxs_ap)
    _out_ap = g.lower_ap(out_ap)
    return g.add_instruction(
        mybir.InstDMAGatherAnt(
            name=g.bass.get_next_instruction_name(),
            ins=[*_in_ap, _idxs_ap, g.lower_val_access(g.to_reg(num_idxs_reg))],
            outs=[_out_ap],
            transpose=False,
            num_idxs=num_idxs,
            elem_size=elem_size,
            stride_bytes_256=stride_bytes_256,
            gen_mode=0,
            single_packet=False,
            queue_num=queue_num,
            sbuf_tokens_per_rank=0,
            sbuf_free_dim_per_rank=0,
            sbuf_free_dim_pad_per_rank=0,
            sbuf_byte_offset=0,
        )
    )


# --------------------------------------------------------------------------
# device programs
# --------------------------------------------------------------------------

def build_l1():
    """ypfm[h, g] = dinv[g] * (x @ W1)[g, h] feature-major -> bf16.

    W1-stationary: lhsT = W1-half [128f, 16], rhs = xT-half quarter
    [128f, 3328 nodes] (8 large DMAs total), out [16, 512] PSUM
    accumulated over the two 128-feature halves. Host transposes the
    feature-major output when building the quad table (free).
    """
    G = 512                       # nodes per matmul group
    QN = PC // 2                  # 6656 nodes per half (13 groups)
    nc = bacc.Bacc()
    xT = nc.declare_dram_parameter("xT", [F, PC], BF16, isOutput=False)
    w1 = nc.declare_dram_parameter("w1", [F, H], BF16, isOutput=False)
    dinvr = nc.declare_dram_parameter("dinvr", [H, PC], F32, isOutput=False)
    yp = nc.declare_dram_parameter("yp", [H, PC], BF16, isOutput=True)

    with tile.TileContext(nc) as tc:
        with (
            tc.tile_pool(name="w", bufs=1) as wp,
            tc.tile_pool(name="x", bufs=2) as xp,
            tc.tile_pool(name="ps", bufs=4, space="PSUM") as pp,
        ):
            w1a = wp.tile([P, H], BF16, tag="w1a")
            w1b = wp.tile([P, H], BF16, tag="w1b")
            dvf = wp.tile([H, PC], F32, tag="dvf")
            yf = wp.tile([H, PC], BF16, tag="yf")
            nc.sync.dma_start(out=w1a[:], in_=w1[0:P, :])
            nc.sync.dma_start(out=w1b[:], in_=w1[P:F, :])
            nc.sync.dma_start(out=dvf[:], in_=dinvr[:, :])

            for q in range(2):
                cols = slice(q * QN, (q + 1) * QN)
                xa = xp.tile([P, QN], BF16, tag="xa")
                xb = xp.tile([P, QN], BF16, tag="xb")
                nc.sync.dma_start(out=xa[:], in_=xT[0:P, cols])
                nc.sync.dma_start(out=xb[:], in_=xT[P:F, cols])
                for t in range(QN // G):
                    g0 = q * QN + t * G
                    ps = pp.tile([H, G], F32, tag="ps")
                    nc.tensor.matmul(out=ps[:], lhsT=w1a[:],
                                     rhs=xa[:, t * G:(t + 1) * G],
                                     start=True, stop=False)
                    nc.tensor.matmul(out=ps[:], lhsT=w1b[:],
                                     rhs=xb[:, t * G:(t + 1) * G],
                                     start=False, stop=True)
                    nc.vector.tensor_tensor(
                        out=yf[:, g0:g0 + G], in0=ps[:],
                        in1=dvf[:, g0:g0 + G], op=ALU.mult)
            nc.sync.dma_start(out=yp[:, :], in_=yf[:])
    nc.compile()
    return nc


def build_gather_layer(schedules, final):
    """Ragged gather-sum layer over the layout-ordered node space.

    schedules: per chunk, list of n_k (multiples of 16; slots covered by
    call k). Self-loop rows come from `selfr` (sequential), so call k
    gathers the (k+1)-th in-neighbor of each covered slot.

    final=False (L2): out = bf16[ dinv * relu(dinv*(agg+self) + b1) ].
    final=True  (L3): out = f32[ (dinv*(agg+self)) @ W2 + b2 ].
    """
    nch = len(schedules)
    # per-chunk cumulative offsets
    woff, moff, cb = [0], [0], [0]
    for ch in range(nch):
        woff.append(woff[-1] + sum(n // 16 for n in schedules[ch]))
        moff.append(moff[-1] + sum(((n + 127) // 128) * 4
                                   for n in schedules[ch]))
        cb.append(cb[-1] + len(schedules[ch]))
    TOTW, TOTM, TOT = woff[-1], moff[-1], cb[-1]
    maxw = max(woff[i + 1] - woff[i] for i in range(nch))
    maxm = max(moff[i + 1] - moff[i] for i in range(nch))

    nc = bacc.Bacc(num_swdge_queues=NQ)
    table = nc.declare_dram_parameter("table", [QROWS, P], BF16,
                                      isOutput=False)
    idxs = nc.declare_dram_parameter("idxs", [P, TOTW], I16, isOutput=False)
    masks = nc.declare_dram_parameter("masks", [P, TOTM], BF16,
                                      isOutput=False)
    dinvp = nc.declare_dram_parameter("dinvp", [P, nch * C], F32,
                                      isOutput=False)
    selfr = nc.declare_dram_parameter("selfr", [PC, H], BF16, isOutput=False)
    if final:
        idn = nc.declare_dram_parameter("idn", [P, P], BF16, isOutput=False)
        w2b = nc.declare_dram_parameter("w2b", [P, C * O], BF16,
                                        isOutput=False)
        b2r = nc.declare_dram_parameter("b2r", [P, C * O], F32, isOutput=False)
        out = nc.declare_dram_parameter("out", [PC, O], F32, isOutput=True)
    else:
        b1r = nc.declare_dram_parameter("b1r", [P, C * H], F32,
                                        isOutput=False)
        out = nc.declare_dram_parameter("out", [PC, H], BF16, isOutput=True)
    OW = O if final else H
    NCONST = 3 if final else 1

    with ExitStack() as st:
        blk = st.enter_context(nc.Block())
        gb = [st.enter_context(nc.sbuf_tensor(f"gb{i}", [P, C * 64], BF16))
              for i in range(B)]
        idxb = [st.enter_context(nc.sbuf_tensor(f"idxb{i}", [P, maxw], I16))
                for i in range(2)]
        maskb = [st.enter_context(nc.sbuf_tensor(f"maskb{i}", [P, maxm],
                                                 BF16)) for i in range(2)]
        dvb = [st.enter_context(nc.sbuf_tensor(f"dvb{i}", [P, C], F32))
               for i in range(2)]
        sfb = [st.enter_context(nc.sbuf_tensor(f"sfb{i}", [P, C * H], BF16))
               for i in range(2)]
        tm = st.enter_context(nc.sbuf_tensor("tm", [P, C * 64], BF16))
        zer = st.enter_context(nc.sbuf_tensor("zer", [P, C * H], F32))
        accw = st.enter_context(nc.sbuf_tensor("accw", [P, C * 64], F32))
        acc = st.enter_context(nc.sbuf_tensor("acc", [P, C * H], F32))
        ob = [st.enter_context(nc.sbuf_tensor(f"ob{i}", [P, C * OW],
                                              F32 if final else BF16))
              for i in range(2)]
        cst = st.enter_context(nc.sbuf_tensor("cst", [P, C * O], F32))
        if final:
            idnb = st.enter_context(nc.sbuf_tensor("idnb", [P, P], BF16))
            w2bb = st.enter_context(nc.sbuf_tensor("w2bb", [P, C * O], BF16))
            accb = st.enter_context(nc.sbuf_tensor("accb", [P, C * H], BF16))
            accTs = st.enter_context(nc.sbuf_tensor("accTs", [P, P], BF16))
            psT = st.enter_context(nc.psum_tensor("psT", [P, P], BF16))
            psO = st.enter_context(nc.psum_tensor("psO", [P, C * O], F32))
        isem = st.enter_context(nc.semaphore("isem"))
        iisem = st.enter_context(nc.semaphore("iisem"))
        tsem = st.enter_context(nc.semaphore("tsem"))
        psem = st.enter_context(nc.semaphore("psem"))
        t2sem = st.enter_context(nc.semaphore("t2sem"))
        p2sem = st.enter_context(nc.semaphore("p2sem"))
        qsems = [[st.enter_context(nc.semaphore(f"qsem{q}_{s}"))
                  for s in range(RS)] for q in range(NQ)]
        csem = st.enter_context(nc.semaphore("csem"))
        hsem = st.enter_context(nc.semaphore("hsem"))
        osem = st.enter_context(nc.semaphore("osem"))

        # chunk-0 idx split point: first 4 calls load first so gathers
        # start before the bulk of chunk 0's indices arrive
        W0A = sum(n // 16 for n in schedules[0][:4])

        @blk.sync
        def _(sp: bass.BassEngine):
            sp.dma_start(idxb[0][:, 0:W0A],
                         idxs[:, 0:W0A]).then_inc(iisem, 16)
            if final:
                sp.dma_start(idnb[:], idn[:, :]).then_inc(isem, 16)
                sp.dma_start(w2bb[:], w2b[:, :]).then_inc(isem, 16)
                sp.dma_start(cst[:, 0:C * O], b2r[:, :]).then_inc(isem, 16)
            else:
                sp.dma_start(cst[:, 0:C * H], b1r[:, :]).then_inc(isem, 16)
            for ch in range(nch):
                if ch >= 2:
                    # idx buf (ch-2)%2 reused: DVE consumed chunk ch-2's calls
                    # implies their gathers (reading idxb) completed
                    sp.wait_ge(csem, int(cb[ch - 1]))
                    # mask/dv/self bufs reused: chunk ch-2 epilogue done
                    sp.wait_ge(hsem, ch - 1)
                sp.dma_start(
                    idxb[ch % 2][:, (W0A if ch == 0 else 0):
                                 woff[ch + 1] - woff[ch]],
                    idxs[:, woff[ch] + (W0A if ch == 0 else 0):woff[ch + 1]],
                ).then_inc(iisem, 16)
                sp.dma_start(
                    maskb[ch % 2][:, 0:moff[ch + 1] - moff[ch]],
                    masks[:, moff[ch]:moff[ch + 1]],
                ).then_inc(isem, 16)
                sp.dma_start(
                    dvb[ch % 2][:],
                    dinvp[:, ch * C:(ch + 1) * C],
                ).then_inc(isem, 16)
                sp.dma_start(
                    sfb[ch % 2][:].rearrange("p (c h) -> p c h", h=H),
                    selfr[ch * CH:(ch + 1) * CH, :]
                    .rearrange("(c p) h -> p c h", p=P),
                ).then_inc(isem, 16)
                if ch >= 1:
                    sp.wait_ge(hsem, ch)
                    prows = slice((ch - 1) * CH, ch * CH)
                    sp.dma_start(
                        out[prows, :].rearrange("(c p) o -> p c o", p=P),
                        ob[(ch - 1) % 2][:].rearrange("p (c o) -> p c o",
                                                      o=OW),
                    ).then_inc(osem, 16)
            sp.wait_ge(hsem, nch)
            prows = slice((nch - 1) * CH, PC)
            sp.dma_start(
                out[prows, :].rearrange("(c p) o -> p c o", p=P),
                ob[(nch - 1) % 2][:].rearrange("p (c o) -> p c o", o=OW),
            ).then_inc(osem, 16)
            sp.wait_ge(osem, 16 * nch)
            sp.wait_ge(csem, TOT)

        @blk.gpsimd
        def _(gp: bass.BassGpSimd):
            for ch in range(nch):
                if ch == 0:
                    gp.wait_ge(iisem, 16)
                else:
                    gp.wait_ge(iisem, 16 * (ch + 2))
                w = 0
                for k, nk in enumerate(schedules[ch]):
                    if ch == 0 and k == 4:
                        gp.wait_ge(iisem, 32)
                    j = int(cb[ch]) + k
                    ck = (nk + 127) // 128
                    if j >= B:
                        gp.wait_ge(csem, j - B + 1)
                    dma_gather_raw(
                        gp,
                        gb[j % B][:, 0:ck * 64].rearrange(
                            "p (c e) -> p c e", e=64),
                        table[:, 0:64],
                        idxb[ch % 2][:, w:w + nk // 16],
                        nk, nk, 64, P,
                        queue_num=j % NQ,
                    ).then_inc(qsems[j % NQ][(j // NQ) % RS], 16)
                    w += nk // 16

        if final:
            @blk.tensor
            def _(t: bass.BassTensorEngine):
                t.wait_ge(isem, 16 * NCONST)
                for ch in range(nch):
                    if ch >= 1:
                        t.wait_ge(t2sem, ch)   # psT consumed by DVE copy
                    t.wait_ge(tsem, ch + 1)    # accb ready
                    t.transpose(out=psT[:], in_=accb[:], identity=idnb[:]) \
                        .then_inc(psem, 1)
                    if ch >= 1:
                        t.wait_ge(hsem, ch)    # psO consumed by bias-add
                    t.wait_ge(t2sem, ch + 1)   # accTs ready
                    t.matmul(
                        out=psO[:], lhsT=accTs[:], rhs=w2bb[:],
                        start=True, stop=True,
                    ).then_inc(p2sem, 1)

        @blk.vector
        def _(v: bass.BassVectorEngine):
            v.memset(zer[:], 0.0)
            for ch in range(nch):
                v.wait_ge(isem, 16 * (NCONST + 3 * (ch + 1)))
                v.memset(accw[:], 0.0)
                m = 0
                for k, nk in enumerate(schedules[ch]):
                    j = int(cb[ch]) + k
                    ck = (nk + 127) // 128
                    v.wait_ge(qsems[j % NQ][(j // NQ) % RS],
                              16 * (j // (NQ * RS) + 1))
                    # tm = g * mask  (mask selects 1 of 4 quad sub-rows and
                    # zeroes rounding-pad slots)
                    g3 = (gb[j % B][:, 0:ck * 64]
                          .rearrange("p (s h) -> p s h", h=H))
                    m3 = (maskb[ch % 2][:, m:m + ck * 4]
                          .rearrange("p (s one) -> p s one", one=1)
                          .to_broadcast([P, ck * 4, H]))
                    v.tensor_tensor(
                        out=tm[:, 0:ck * 64].rearrange("p (s h) -> p s h",
                                                       h=H),
                        in0=g3, in1=m3, op=ALU.mult,
                    ).then_inc(csem, 1)
                    v.tensor_tensor(out=accw[:, 0:ck * 64],
                                    in0=accw[:, 0:ck * 64],
                                    in1=tm[:, 0:ck * 64], op=ALU.add)
                    m += ck * 4
                # acc[p, c, h] = sum_q accw[p, c, q, h] + self
                v.tensor_reduce(
                    out=acc[:].rearrange("p (c h) -> p c h", h=H),
                    in_=accw[:].rearrange("p (c q h) -> p c h q", q=4, h=H),
                    axis=AXL.X, op=ALU.add,
                )
                v.tensor_tensor(out=acc[:], in0=acc[:], in1=sfb[ch % 2][:],
                                op=ALU.add)
                # post: scale by dinv etc.
                acc3 = acc[:].rearrange("p (c h) -> p c h", h=H)
                dv3 = dvb[ch % 2][:].unsqueeze(2).to_broadcast([P, C, H])
                if final:
                    accb3 = accb[:].rearrange("p (c h) -> p c h", h=H)
                    v.tensor_tensor(out=accb3, in0=acc3, in1=dv3,
                                    op=ALU.mult).then_inc(tsem, 1)
                else:
                    v.tensor_tensor(out=acc3, in0=acc3, in1=dv3, op=ALU.mult)
                if ch >= 2:
                    v.wait_ge(osem, 16 * (ch - 1))
                o3 = ob[ch % 2][:].rearrange("p (c o) -> p c o", o=OW)
                if final:
                    # PE transposes accb and applies block-diagonal W2
                    v.wait_ge(psem, ch + 1)
                    v.tensor_copy(out=accTs[:], in_=psT[:]) \
                        .then_inc(t2sem, 1)
                    v.wait_ge(p2sem, ch + 1)
                    b23 = cst[:, 0:C * O].rearrange("p (c o) -> p c o", o=O)
                    ps3 = psO[:].rearrange("p (c o) -> p c o", o=O)
                    v.tensor_tensor(out=o3, in0=ps3, in1=b23,
                                    op=ALU.add).then_inc(hsem, 1)
                else:
                    b13 = cst[:, 0:C * H].rearrange("p (c h) -> p c h", h=H)
                    v.tensor_tensor(out=acc3, in0=acc3, in1=b13, op=ALU.add)
                    v.tensor_tensor(out=acc[:], in0=acc[:], in1=zer[:],
                                    op=ALU.max)
                    v.tensor_tensor(out=o3, in0=acc3, in1=dv3,
                                    op=ALU.mult).then_inc(hsem, 1)
    nc.compile()
    return nc


# --------------------------------------------------------------------------
# host orchestration
# --------------------------------------------------------------------------

def _install_trace_shim():
    import types
    import contextlib
    import ctypes

    if "antenv.axon_hooks" not in sys.modules:
        lib = ctypes.CDLL("/opt/axon/libaxon_pjrt.so")
        lib.axon_start_nrt_profile.argtypes = [
            ctypes.POINTER(ctypes.c_int64), ctypes.c_size_t]
        lib.axon_start_nrt_profile.restype = ctypes.c_int64
        lib.axon_stop_nrt_profile.argtypes = [ctypes.c_char_p]
        lib.axon_stop_nrt_profile.restype = ctypes.c_int64

        @contextlib.contextmanager
        def _hook(output_dir, device_ids):
            import jax
            jax.devices()
            if device_ids:
                ids = (ctypes.c_int64 * len(device_ids))(*device_ids)
                rc = lib.axon_start_nrt_profile(ids, len(device_ids))
            else:
                rc = lib.axon_start_nrt_profile(None, 0)
            if rc != 0:
                raise RuntimeError(f"axon_start_nrt_profile rc={rc}")
            try:
                yield
            finally:
                n = lib.axon_stop_nrt_profile(str(output_dir).encode())
                print(f"profile: {n} file(s) -> {output_dir}", file=sys.stderr)

        mod = types.ModuleType("antenv.axon_hooks")
        mod.get_axon_ntff_profile_hook = lambda: _hook
        mod.set_axon_ntff_profile_hook = lambda h: None
        sys.modules["antenv.axon_hooks"] = mod

    import concourse.bass_utils as bu
    bu.upload_artifacts = lambda tmpdir: "local://skipped"


def _run(nc, in_maps, label):
    if _TRACE:
        _install_trace_shim()
        res = run_bass_kernel_spmd(
            nc, in_maps, list(range(NCORES)), trace=True, trace_cores=[0],
        )
    else:
        res = run_bass_kernel_spmd(nc, in_maps, list(range(NCORES)))
    if res.exec_time_ns is not None:
        print(f"[{label}] exec_time_ns={res.exec_time_ns}", file=sys.stderr)
        _EXEC_NS.append((label, res.exec_time_ns))
    if res.instructions_and_trace is not None:
        print(f"[{label}] trace={res.instructions_and_trace[1]}",
              file=sys.stderr)
    return res.results


def _quad_table(rows16):
    """[NPAD, 16] bf16 -> padded quad table [QROWS, 128] bf16 (cols 64+ 0)."""
    t = np.zeros((QROWS, P), dtype=ml_dtypes.bfloat16)
    t[:, 0:64] = np.asarray(rows16).reshape(QROWS, 64)
    return t


def _roundup16(x):
    return -(-int(x) // 16) * 16


def prep(edge_index):
    """All graph-dependent host prep; returns dict of per-core arrays."""
    src = np.ascontiguousarray(edge_index[0]).astype(np.int64)
    dst = np.ascontiguousarray(edge_index[1]).astype(np.int64)
    E = src.shape[0]

    counts = np.bincount(dst, minlength=NPAD).astype(np.int64)
    dinv = np.zeros(NPAD, np.float32)
    dinv[:N] = 1.0 / np.sqrt((counts[:N] + 1).astype(np.float64))

    order_e = np.argsort(dst, kind="stable")
    src_sorted = src[order_e].astype(np.int64)
    starts = np.zeros(NPAD + 1, np.int64)
    np.cumsum(counts, out=starts[1:])

    ordern = np.argsort(-counts, kind="stable").astype(np.int64)
    blocks = ordern.reshape(CHUNKS, CH * NCORES)
    node_layout = blocks.reshape(CHUNKS, CH, NCORES).transpose(2, 0, 1)
    layout_nodes = node_layout.reshape(-1)          # table row g -> node id
    gpos = np.empty(NPAD, np.int64)
    gpos[layout_nodes] = np.arange(NPAD)

    cnts_l = counts[node_layout]                    # [core, ch, i] desc order

    # conjugate staircase schedule, shared across cores (SPMD)
    schedules = []
    for ch in range(CHUNKS):
        K = int(cnts_l[:, ch, :].max())
        nks = []
        for k in range(K):
            nv = int((cnts_l[:, ch, :] > k).sum(axis=1).max())
            if nv == 0:
                break
            nks.append(_roundup16(nv))
        schedules.append(nks)

    woffs = [sum(n // 16 for n in nks) for nks in schedules]
    moffs = [sum(((n + 127) // 128) * 4 for n in nks) for nks in schedules]
    TOTW, TOTM = sum(woffs), sum(moffs)

    idx_cores, mask_cores, dinvp_cores = [], [], []
    for core in range(NCORES):
        idx_arr = np.full((P, TOTW), ZRQ, np.int16)
        mask_arr = np.zeros((P, TOTM), ml_dtypes.bfloat16)
        w = m = 0
        for ch in range(CHUNKS):
            nodes = node_layout[core, ch]
            cnt = cnts_l[core, ch]
            nks = schedules[ch]
            K = len(nks)
            kk = np.arange(K, dtype=np.int64)
            pos = starts[nodes][:, None] + kk[None, :]
            valid = kk[None, :] < cnt[:, None]
            gv = np.where(valid,
                          gpos[src_sorted[np.clip(pos, 0, E - 1)]],
                          4 * ZRQ)                   # [1024, K]
            for k, nk in enumerate(nks):
                v = gv[:nk, k]
                idx16 = (v >> 2).astype(np.int16)
                idx_arr[:, w:w + nk // 16] = np.tile(
                    idx16.reshape(nk // 16, 16).T, (8, 1))
                ck = (nk + 127) // 128
                mm = np.zeros((ck * 128, 4), np.float32)
                mm[np.arange(nk), v & 3] = valid[:nk, k]
                mask_arr[:, m:m + ck * 4] = (
                    mm.reshape(ck, 128, 4).transpose(1, 0, 2)
                    .reshape(128, ck * 4))
                w += nk // 16
                m += ck * 4
        idx_cores.append(idx_arr)
        mask_cores.append(np.ascontiguousarray(mask_arr))
        dvi = dinv[node_layout[core].reshape(-1)]    # [PC] layout order
        dinvp_cores.append(np.ascontiguousarray(
            dvi.reshape(CHUNKS, C, P).transpose(2, 0, 1).reshape(P, -1)))

    return dict(
        dinv=dinv, schedules=schedules, idx=idx_cores, mask=mask_cores,
        dinvp=dinvp_cores, layout_nodes=layout_nodes,
        counts=counts, starts=starts, src_sorted=src_sorted,
        node_layout=node_layout,
    )


def kernel(x, edge_index, W1, b1, W2, b2):
    x = np.ascontiguousarray(np.asarray(x, dtype=np.float32))
    W1 = np.ascontiguousarray(np.asarray(W1, dtype=np.float32))
    b1 = np.asarray(b1, dtype=np.float32).reshape(-1)
    W2 = np.ascontiguousarray(np.asarray(W2, dtype=np.float32))
    b2 = np.asarray(b2, dtype=np.float32).reshape(-1)

    pp = prep(np.asarray(edge_index))
    dinv, schedules = pp["dinv"], pp["schedules"]
    layout_nodes = pp["layout_nodes"]

    # layout-ordered, pre-scaled, bf16 inputs for L1
    xpad = np.zeros((NPAD, F), np.float32)
    xpad[:N] = x
    xl = xpad[layout_nodes]                          # [NPAD, F] layout order
    dinv_l = dinv[layout_nodes]

    b1r = np.ascontiguousarray(np.tile(b1[None, :], (P, C)))
    b2r = np.ascontiguousarray(np.tile(b2[None, :], (P, C)))
    idn = np.eye(P, dtype=ml_dtypes.bfloat16)
    w2b = np.zeros((P, C * O), np.float32)
    for c in range(C):
        w2b[c * H:(c + 1) * H, c * O:(c + 1) * O] = W2
    w2b = np.ascontiguousarray(w2b).astype(ml_dtypes.bfloat16)

    # ---- L1 ----
    nc1 = build_l1()
    maps1 = [
        {
            "xT": np.ascontiguousarray(
                xl[core * PC:(core + 1) * PC].T).astype(ml_dtypes.bfloat16),
            "w1": W1.astype(ml_dtypes.bfloat16),
            "dinvr": np.ascontiguousarray(np.tile(
                dinv_l[None, core * PC:(core + 1) * PC], (H, 1))),
        }
        for core in range(NCORES)
    ]
    r1 = _run(nc1, maps1, "L1")
    ypl = np.concatenate(
        [np.ascontiguousarray(np.asarray(r1[i]["yp"]).T)
         for i in range(NCORES)], axis=0)
    tbl1 = _quad_table(ypl)

    # ---- L2 ----
    nc2 = build_gather_layer(schedules, final=False)
    maps2 = [
        {"table": tbl1, "idxs": pp["idx"][core], "masks": pp["mask"][core],
         "dinvp": pp["dinvp"][core],
         "selfr": np.ascontiguousarray(ypl[core * PC:(core + 1) * PC]),
         "b1r": b1r}
        for core in range(NCORES)
    ]
    r2 = _run(nc2, maps2, "L2")
    hl = np.concatenate(
        [np.asarray(r2[i]["out"]) for i in range(NCORES)], axis=0)
    tbl2 = _quad_table(hl)

    # ---- L3 ----
    nc3 = build_gather_layer(schedules, final=True)
    maps3 = [
        {"table": tbl2, "idxs": pp["idx"][core], "masks": pp["mask"][core],
         "dinvp": pp["dinvp"][core],
         "selfr": np.ascontiguousarray(hl[core * PC:(core + 1) * PC]),
         "idn": idn, "w2b": w2b, "b2r": b2r}
        for core in range(NCORES)
    ]
    r3 = _run(nc3, maps3, "L3")
    outl = np.concatenate(
        [np.asarray(r3[i]["out"]) for i in range(NCORES)], axis=0)
    outp = np.zeros((NPAD, O), np.float32)
    outp[layout_nodes] = outl
    return np.ascontiguousarray(outp[:N])
